# revision 31
# baseline (speedup 1.0000x reference)
"""GAT layer (single head, PyG GATConv semantics + relu) on 8 Trainium2 cores.

Strategy (destination-major, descriptor-minimized):
  * ONE feature table in HBM per core (replicated build): row r = 256B =
    [h bf16 x64 | a_src f32 | pad], rows ordered by FIRST USE so each
    superchunk's gathers only need a table PREFIX -> the table build (phase
    A) overlaps the edge gathers (phase B).  Rows 0 and MID=17408 are pad
    rows (a_src = -1e4 => exp() = 0 for padded edge slots).
  * Two gather windows (int16 indices span 32768 rows): A = rows [0, 32768),
    B = rows [17408, 50176).  Rows in the overlap may be fetched by either
    call; per-edge assignment balances each destination's A/B column counts
    (near-zero split padding).  Each window's columns are further split into
    ~14-col calls spread over the 4 SWDGE queues.
  * Destinations sorted by (dA, dB) (fixed-point), dealt node-round-robin to
    cores so per-slot shapes are SPMD-uniform and tight.
  * Self-loops are folded in locally from ownxt (never gathered).
  * Per-slot compute: exp via per-partition-bias activation is replaced by
    superchunk-fused ops (z-add, 2 exps, max, cast, weighted multiply) with
    only the segment reductions per slot.
"""

import ml_dtypes
import numpy as np

import concourse.bass as bass
import concourse.tile as tile
from concourse import bacc, mybir
from concourse.bass_utils import run_bass_kernel_spmd

P = 128
NCORES = 8
TROWS = 50176
MID = 17408
WINB = 32768
NEG_SLOPE = 0.2
PAD_ASRC = -1.0e4
F_OUT = 64
WCOLS = F_OUT + 2


# --------------------------------------------------------------------------
# host-side preprocessing
# --------------------------------------------------------------------------

def _preprocess(edge_index, n_nodes, n_iter=6, cmax=56):
    src = np.asarray(edge_index[0], dtype=np.int64)
    dst = np.asarray(edge_index[1], dtype=np.int64)
    deg = np.bincount(dst, minlength=n_nodes)
    slots = (n_nodes + P * NCORES - 1) // (P * NCORES)

    order = np.argsort(-deg, kind="stable")
    for it in range(n_iter):
        pos = np.empty(n_nodes, np.int64)
        pos[order] = np.arange(n_nodes)
        slot_of = pos // (P * NCORES)
        e_slot = slot_of[dst]
        first = np.full(n_nodes, slots, np.int64)
        np.minimum.at(first, src, e_slot)
        used = first < slots
        uorder = np.argsort(first[used], kind="stable")
        un = used.nonzero()[0][uorder]
        rank = np.full(n_nodes, -1, np.int64)
        rank[un] = np.arange(len(un))
        prank = np.where(rank < MID - 1, rank + 1, rank + 2)
        # group-wise partition-major row remap: build writes 2KB-contiguous
        # runs of 8 rows per partition; 1024-aligned windows are preserved.
        pl = prank % 1024
        pos_t = (prank // 1024) * 1024 + (pl % 128) * 8 + pl // 128
        r = pos_t[src]
        onlyA = r < MID + 1
        onlyB = r >= WINB
        nA = np.bincount(dst[onlyA], minlength=n_nodes)
        nB = np.bincount(dst[onlyB], minlength=n_nodes)
        nM = deg - nA - nB
        tgt = (deg + 1) // 2
        dA = np.clip(tgt, nA, nA + nM)
        dB = deg - dA
        if it < n_iter - 1:
            order = np.lexsort((dB, dA))[::-1].copy()

    cap = slots * NCORES * P
    na = np.full(cap, -1, np.int64)
    na[:n_nodes] = order
    # sorted node r -> core r%8, slot r//1024, p (r//8)%128
    node_at = np.transpose(na.reshape(slots, P, NCORES), (2, 0, 1)).copy()

    da = np.zeros(cap, np.int64)
    db = np.zeros(cap, np.int64)
    da[:n_nodes] = dA[order]
    db[:n_nodes] = dB[order]
    D_A = da.reshape(slots, -1).max(axis=1)
    D_B = db.reshape(slots, -1).max(axis=1)

    prefA = np.full(slots, 2, np.int64)
    prefB = np.full(slots, MID + 2, np.int64)
    mA = r < WINB
    np.maximum.at(prefA, e_slot[mA], r[mA] + 1)
    mB = r >= MID + 1
    np.maximum.at(prefB, e_slot[mB], r[mB] + 1)
    prefA = np.maximum.accumulate(np.minimum(prefA, WINB))
    prefB = np.maximum.accumulate(prefB)
    prefA = np.minimum(-(-prefA // 1024) * 1024, WINB)
    prefB = np.minimum(-(-prefB // 1024) * 1024, TROWS)

    scs = []
    cur, cur_c = [], 0
    for s in range(slots):
        c = int(D_A[s] + D_B[s])
        single = s < 4 or s >= slots - 2
        if cur and (single or cur_c + c > cmax):
            scs.append(cur)
            cur, cur_c = [], 0
        cur.append(s)
        cur_c += c
        if single:
            scs.append(cur)
            cur, cur_c = [], 0
    if cur:
        scs.append(cur)

    return dict(
        deg=deg, dA=dA, dB=dB, order=order, pos=pos, pos_t=pos_t, rank=rank,
        prank=prank, node_at=node_at, slots=slots, D_A=D_A, D_B=D_B,
        prefA=prefA, prefB=prefB, scs=scs, src=src, dst=dst,
        n_nodes=n_nodes,
    )


def _build_gather_lists(meta):
    """Per (core, sc): (gA [P, colsA], gB [P, colsB]) window-local rows."""
    src, dst, pos = meta["src"], meta["dst"], meta["pos"]
    pos_t = meta["pos_t"]
    dA = meta["dA"]
    D_A, D_B = meta["D_A"], meta["D_B"]
    slots = meta["slots"]
    n_nodes = meta["n_nodes"]

    r_node = pos[dst]
    core_of = r_node % NCORES
    sp = r_node // NCORES
    slot_of = sp // P
    p_of = sp % P

    r = pos_t[src]
    onlyB = r >= WINB
    onlyA = r < MID + 1
    midm = ~onlyA & ~onlyB

    eo = np.lexsort((np.arange(len(src)), dst))
    is_mid = midm[eo]
    dsts = dst[eo]
    midrank = np.zeros(len(eo), np.int64)
    key = dsts[is_mid]
    grp_start = np.zeros(n_nodes + 1, np.int64)
    np.add.at(grp_start[1:], key, 1)
    np.cumsum(grp_start, out=grp_start)
    midrank[is_mid] = np.arange(is_mid.sum()) - grp_start[key]
    nA_map = np.bincount(dst[onlyA], minlength=n_nodes)
    quota = dA - nA_map
    toA = np.zeros(len(eo), bool)
    toA[~is_mid] = onlyA[eo][~is_mid]
    toA[is_mid] = midrank[is_mid] < quota[key]

    ek = core_of[eo]
    es = slot_of[eo]
    ep = p_of[eo]
    er = r[eo]
    skey = dsts * 2 + (~toA).astype(np.int64)
    sord = np.lexsort((np.arange(len(eo)), skey))
    _, first_idx, counts = np.unique(skey[sord], return_index=True,
                                     return_counts=True)
    jj = np.empty(len(eo), np.int64)
    jj[sord] = np.arange(len(eo)) - np.repeat(first_idx, counts)

    sc_of_slot = np.empty(slots, np.int64)
    bi_of_slot = np.empty(slots, np.int64)
    for ci, sc in enumerate(meta["scs"]):
        for bi, s in enumerate(sc):
            sc_of_slot[s] = ci
            bi_of_slot[s] = bi
    sc_offs = []
    for sc in meta["scs"]:
        offA = np.concatenate([[0], np.cumsum(D_A[sc])])
        offB = np.concatenate([[0], np.cumsum(D_B[sc])])
        sc_offs.append((offA.astype(int), offB.astype(int)))

    eci = sc_of_slot[es]
    ebi = bi_of_slot[es]
    lists = {}
    for ci, sc in enumerate(meta["scs"]):
        offA, offB = sc_offs[ci]
        for k in range(NCORES):
            lists[(k, ci)] = (np.zeros((P, int(offA[-1])), np.int64),
                              np.zeros((P, int(offB[-1])), np.int64))
    colA_e = np.zeros(len(eo), np.int64)
    colB_e = np.zeros(len(eo), np.int64)
    for ci in range(len(meta["scs"])):
        offA, offB = sc_offs[ci]
        m = eci == ci
        colA_e[m] = offA[ebi[m]] + jj[m]
        colB_e[m] = offB[ebi[m]] + jj[m]
    for k in range(NCORES):
        mk = ek == k
        ma = mk & toA
        mb = mk & ~toA
        for ci in range(len(meta["scs"])):
            gA, gB = lists[(k, ci)]
            mm = ma & (eci == ci)
            gA[ep[mm], colA_e[mm]] = er[mm]
            mm = mb & (eci == ci)
            gB[ep[mm], colB_e[mm]] = er[mm] - MID
    return lists, sc_offs


def _wrap_idx(arr):
    """dma_gather index layout: [128, n/16] int16, idx i at (i%16, i//16),
    replicated across the 8 Q7 core groups."""
    n = arr.shape[0]
    assert n % 16 == 0
    w = arr.reshape(n // 16, 16).T.astype(np.int16)
    return np.tile(w, (8, 1))


def _plan_calls(meta, lists, chunk_cols=14):
    """Split each sc's A/B column ranges into calls, balance over 4 queues.

    Returns: calls (list of dicts), gidx per core [P, gc16]."""
    calls = []
    qload = [0, 0, 0, 0]
    off16 = 0
    gidx = [[] for _ in range(NCORES)]
    for ci, sc in enumerate(meta["scs"]):
        gA0, gB0 = lists[(0, ci)]
        for side, cols, pref in (
            (0, gA0.shape[1], int(meta["prefA"][sc[-1]])),
            (1, gB0.shape[1], int(meta["prefB"][sc[-1]])),
        ):
            if cols == 0:
                continue
            nch = max(1, -(-cols // chunk_cols))
            bounds = np.linspace(0, cols, nch + 1).astype(int)
            for c0, c1 in zip(bounds[:-1], bounds[1:]):
                if c1 == c0:
                    continue
                q = min(range(4), key=lambda i: qload[i])
                qload[q] += c1 - c0
                ln16 = (c1 - c0) * P // 16
                for k in range(NCORES):
                    g = lists[(k, ci)][side]
                    gidx[k].append(_wrap_idx(g[:, c0:c1].T.ravel()))
                calls.append(dict(ci=ci, side=side, c0=int(c0), c1=int(c1),
                                  pref=pref, q=q, off16=off16, ln16=ln16))
                off16 += ln16
    gidx = [np.concatenate(g, axis=1) if g else np.zeros((P, 16), np.int16)
            for g in gidx]
    return calls, gidx, off16


# --------------------------------------------------------------------------
# device program
# --------------------------------------------------------------------------

def _build_nc(cfg):
    slots = cfg["slots"]
    scs = cfg["scs"]
    sc_offs = cfg["sc_offs"]
    D_A, D_B = cfg["D_A"], cfg["D_B"]
    calls = cfg["calls"]
    gc16 = max(cfg["gc16"], 16)

    fp32 = mybir.dt.float32
    bf16 = mybir.dt.bfloat16

    nc = bacc.Bacc("TRN2", target_bir_lowering=False, debug=False,
                   num_devices=NCORES, num_swdge_queues=4)
    xTb = nc.dram_tensor("xTb", [P, TROWS], bf16, kind="ExternalInput")
    wextb = nc.dram_tensor("wextb", [P, WCOLS], bf16, kind="ExternalInput")
    ownxt = nc.dram_tensor("ownxt", [P, slots * P], bf16,
                           kind="ExternalInput")
    gidx_d = nc.dram_tensor("gidx", [P, gc16], mybir.dt.int16,
                            kind="ExternalInput")
    biasb = nc.dram_tensor("biasb", [P, F_OUT], fp32, kind="ExternalInput")
    padrow = nc.dram_tensor("padrow", [1, P], bf16, kind="ExternalInput")
    out_d = nc.dram_tensor("out", [slots * P, F_OUT], fp32,
                           kind="ExternalOutput")
    tbl = nc.dram_tensor("tbl", [TROWS, P], bf16, kind="Internal")

    with tile.TileContext(nc) as tc:
        with (
            tc.tile_pool(name="const", bufs=1) as cpool,
            tc.tile_pool(name="xt", bufs=4) as xtpool,
            tc.tile_pool(name="ps", bufs=7, space="PSUM") as pspool,
            tc.tile_pool(name="tstage", bufs=4) as tspool,
            tc.tile_pool(name="gat", bufs=5) as gpool,
            tc.tile_pool(name="wgt", bufs=3) as wpool,
            tc.tile_pool(name="sc", bufs=3) as scpool,
            tc.tile_pool(name="blk", bufs=3) as bpool,
        ):
            wextb_sb = cpool.tile([P, WCOLS], bf16)
            nc.sync.dma_start(out=wextb_sb[:], in_=wextb[:])
            biasb_sb = cpool.tile([P, F_OUT], fp32)
            nc.sync.dma_start(out=biasb_sb[:], in_=biasb[:])
            ownxt_sb = cpool.tile([P, slots * P], bf16)
            gidx_sb = cpool.tile([P, gc16], mybir.dt.int16)

            h_own = cpool.tile([P, slots, F_OUT], bf16)
            aos = cpool.tile([P, slots], fp32)
            aod = cpool.tile([P, slots], fp32)

            def own_precompute():
                for i0 in range(0, slots, 4):
                    hn = min(4, slots - i0)
                    ps2 = pspool.tile([P, 4, WCOLS], fp32, tag="mm")
                    for j in range(hn):
                        nc.tensor.matmul(
                            out=ps2[:, j, :],
                            lhsT=ownxt_sb[:, (i0 + j) * P:(i0 + j + 1) * P],
                            rhs=wextb_sb[:], start=True, stop=True)
                    if (i0 // 4) % 2 == 0:
                        nc.scalar.copy(out=h_own[:, i0:i0 + hn, :],
                                       in_=ps2[:, 0:hn, 0:F_OUT])
                    else:
                        nc.vector.tensor_copy(out=h_own[:, i0:i0 + hn, :],
                                              in_=ps2[:, 0:hn, 0:F_OUT])
                    nc.vector.tensor_copy(
                        out=aos[:, i0:i0 + hn],
                        in_=ps2[:, 0:hn, F_OUT:F_OUT + 1].rearrange(
                            "p h one -> p (h one)"))
                    nc.vector.tensor_copy(
                        out=aod[:, i0:i0 + hn],
                        in_=ps2[:, 0:hn, F_OUT + 1:F_OUT + 2].rearrange(
                            "p h one -> p (h one)"))
                zown = cpool.tile([P, slots], fp32)
                nc.vector.tensor_tensor(out=zown[:], in0=aos[:], in1=aod[:],
                                        op=mybir.AluOpType.add)
                e1o = cpool.tile([P, slots], fp32)
                e2o = cpool.tile([P, slots], fp32)
                nc.scalar.activation(out=e1o[:], in_=zown[:],
                                     func=mybir.ActivationFunctionType.Exp,
                                     scale=1.0)
                nc.scalar.activation(out=e2o[:], in_=zown[:],
                                     func=mybir.ActivationFunctionType.Exp,
                                     scale=NEG_SLOPE)
                s_ii = cpool.tile([P, slots], fp32)
                nc.vector.tensor_tensor(out=s_ii[:], in0=e1o[:], in1=e2o[:],
                                        op=mybir.AluOpType.max)
                s_ii16 = cpool.tile([P, slots], bf16)
                nc.vector.tensor_copy(out=s_ii16[:], in_=s_ii[:])
                aod02 = cpool.tile([P, slots], fp32)
                nc.vector.tensor_scalar_mul(aod02[:], aod[:], NEG_SLOPE)
                return s_ii, s_ii16, aod02

            # ---- phase A: table build (first-use row order), own-node
            # precompute interleaved after the early prefix groups ----
            WB = 8
            nblk_tbl = TROWS // P
            own_result = [None]
            for g0 in range(0, nblk_tbl, WB):
                gn = min(WB, nblk_tbl - g0)
                xtb8 = xtpool.tile([P, WB, P], bf16, tag="xtb")
                nc.sync.dma_start(
                    out=xtb8[:, 0:gn, :],
                    in_=xTb[:, g0 * P:(g0 + gn) * P].rearrange(
                        "p (i q) -> p i q", q=P))
                tstage = tspool.tile([P, WB, P], bf16)
                for h0 in range(0, gn, 4):
                    hn = min(4, gn - h0)
                    ps4 = pspool.tile([P, 4, WCOLS], fp32, tag="mm")
                    for bi in range(hn):
                        nc.tensor.matmul(out=ps4[:, bi, :],
                                         lhsT=xtb8[:, h0 + bi, :].squeeze(),
                                         rhs=wextb_sb[:],
                                         start=True, stop=True)
                    if h0 == 0:
                        nc.scalar.copy(out=tstage[:, h0:h0 + hn, 0:F_OUT],
                                       in_=ps4[:, 0:hn, 0:F_OUT])
                    else:
                        nc.vector.tensor_copy(out=tstage[:, h0:h0 + hn, 0:F_OUT],
                                              in_=ps4[:, 0:hn, 0:F_OUT])
                    nc.vector.tensor_copy(
                        out=tstage[:, h0:h0 + hn, F_OUT:F_OUT + 4].bitcast(fp32),
                        in_=ps4[:, 0:hn, F_OUT:F_OUT + 2])
                r0, r1 = g0 * P, (g0 + gn) * P
                # partition-major rows: row r0 + p*gn + i <- tstage[p, i, :]
                nc.scalar.dma_start(
                    out=tbl[r0:r1, :].rearrange("(p i) w -> p i w", i=gn),
                    in_=tstage[:, 0:gn, :])
                if r0 == 0:
                    nc.sync.dma_start(out=tbl[0:1, :], in_=padrow[:])
                if r0 <= MID < r1:
                    nc.sync.dma_start(out=tbl[MID:MID + 1, :], in_=padrow[:])
                if g0 == WB:
                    nc.sync.dma_start(out=gidx_sb[:], in_=gidx_d[:])
                if g0 == 18 * WB:
                    nc.sync.dma_start(out=ownxt_sb[:], in_=ownxt[:])
                if g0 == 21 * WB:
                    own_result[0] = own_precompute()
            s_ii, s_ii16, aod02 = own_result[0]

            # ---- phase B ----
            calls_by_sc = {}
            for cl in calls:
                calls_by_sc.setdefault(cl["ci"], []).append(cl)

            for ci, sc in enumerate(scs):
                offA, offB = sc_offs[ci]
                cA, cB = int(offA[-1]), int(offB[-1])
                ncols = cA + cB
                nb = len(sc)
                i0 = sc[0]
                g_t = gpool.tile([P, ncols, P], bf16)
                for cl in calls_by_sc[ci]:
                    base = cl["c0"] + (cA if cl["side"] else 0)
                    n_i = (cl["c1"] - cl["c0"]) * P
                    in_ap = (tbl[0:cl["pref"], :] if cl["side"] == 0
                             else tbl[MID:cl["pref"], :])
                    nc.gpsimd.dma_gather(
                        out_ap=g_t[:, base:base + (cl["c1"] - cl["c0"]), :],
                        in_ap=in_ap,
                        idxs_ap=gidx_sb[:, cl["off16"]:cl["off16"] + cl["ln16"]],
                        num_idxs=n_i, num_idxs_reg=n_i,
                        elem_size=P, single_packet=False, queue_num=cl["q"])

                # per (slot, half): exp with per-partition bias on ACT, then
                # fused max + bf16 cast + denominator-accumulate on DVE
                e1 = scpool.tile([P, ncols], fp32, tag="e1")
                e2 = scpool.tile([P, ncols], fp32, tag="e2")
                s16 = scpool.tile([P, ncols], bf16, tag="s16")
                dn = bpool.tile([P, 2 * nb], fp32, tag="dn")
                asrc_v = g_t[:, :, F_OUT:F_OUT + 2].bitcast(fp32)
                for bi, s in enumerate(sc):
                    for hi, (c0, c1) in enumerate(
                        ((int(offA[bi]), int(offA[bi + 1])),
                         (cA + int(offB[bi]), cA + int(offB[bi + 1])))):
                        j = 2 * bi + hi
                        if c1 > c0:
                            av = asrc_v[:, c0:c1, :].rearrange(
                                "p c one -> p (c one)")
                            nc.scalar.activation(
                                out=e1[:, c0:c1], in_=av,
                                func=mybir.ActivationFunctionType.Exp,
                                bias=aod[:, s:s + 1], scale=1.0)
                            nc.scalar.activation(
                                out=e2[:, c0:c1], in_=av,
                                func=mybir.ActivationFunctionType.Exp,
                                bias=aod02[:, s:s + 1], scale=NEG_SLOPE)
                            nc.vector.scalar_tensor_tensor(
                                out=s16[:, c0:c1], in0=e1[:, c0:c1],
                                scalar=1.0, in1=e2[:, c0:c1],
                                op0=mybir.AluOpType.bypass,
                                op1=mybir.AluOpType.max,
                                accum_out=dn[:, j:j + 1])
                        else:
                            nc.vector.memset(dn[:, j:j + 1], 0.0)
                # weighted features transposed [P, F_OUT, ncols] so the
                # per-slot segment reduces read contiguously
                wgt = wpool.tile([P, F_OUT, ncols], bf16)
                nc.vector.tensor_tensor(
                    out=wgt[:],
                    in0=g_t[:, :, 0:F_OUT].rearrange("p c f -> p f c"),
                    in1=s16[:].unsqueeze(1).broadcast_to([P, F_OUT, ncols]),
                    op=mybir.AluOpType.mult)

                wsA = bpool.tile([P, nb, F_OUT], fp32, tag="wsA")
                wsB = bpool.tile([P, nb, F_OUT], fp32, tag="wsB")
                for bi, s in enumerate(sc):
                    for ws, (c0, c1) in (
                        (wsA, (int(offA[bi]), int(offA[bi + 1]))),
                        (wsB, (cA + int(offB[bi]), cA + int(offB[bi + 1])))):
                        if c1 > c0:
                            nc.vector.tensor_reduce(
                                out=ws[:, bi, :],
                                in_=wgt[:, :, c0:c1],
                                axis=mybir.AxisListType.X,
                                op=mybir.AluOpType.add)
                        else:
                            nc.vector.memset(ws[:, bi, :], 0.0)

                den = bpool.tile([P, nb], fp32, tag="den")
                nc.vector.tensor_reduce(
                    out=den[:], in_=dn[:].rearrange("p (b t) -> p b t", t=2),
                    axis=mybir.AxisListType.X, op=mybir.AluOpType.add)
                nc.vector.tensor_add(den[:], den[:], s_ii[:, i0:i0 + nb])
                rec = bpool.tile([P, nb], fp32, tag="rec")
                nc.vector.reciprocal(rec[:], den[:])

                num = bpool.tile([P, nb, F_OUT], fp32, tag="num")
                nc.vector.tensor_add(num[:], wsA[:], wsB[:])
                selfm = bpool.tile([P, nb, F_OUT], fp32, tag="selfm")
                nc.vector.tensor_tensor(
                    out=selfm[:], in0=h_own[:, i0:i0 + nb, :],
                    in1=s_ii16[:, i0:i0 + nb].unsqueeze(2).broadcast_to(
                        [P, nb, F_OUT]),
                    op=mybir.AluOpType.mult)
                nc.vector.tensor_add(num[:], num[:], selfm[:])
                nc.vector.tensor_tensor(
                    out=num[:], in0=num[:],
                    in1=rec[:].unsqueeze(2).broadcast_to([P, nb, F_OUT]),
                    op=mybir.AluOpType.mult)
                nc.vector.tensor_tensor(
                    out=num[:], in0=num[:],
                    in1=biasb_sb[:].unsqueeze(1).broadcast_to([P, nb, F_OUT]),
                    op=mybir.AluOpType.add)
                ostage = bpool.tile([P, nb, F_OUT], fp32, tag="ostage")
                nc.scalar.activation(out=ostage[:], in_=num[:],
                                     func=mybir.ActivationFunctionType.Relu)
                nc.sync.dma_start(
                    out=out_d[i0 * P:(i0 + nb) * P, :].rearrange(
                        "(i p) f -> p i f", p=P),
                    in_=ostage[:])
    nc.compile()
    return nc


# --------------------------------------------------------------------------
# entry point
# --------------------------------------------------------------------------

_RUN_KW = {}
_LAST_RESULT = [None]


def kernel(x, edge_index, W, att_src, att_dst, bias):
    x = np.asarray(x, dtype=np.float32)
    W = np.asarray(W, dtype=np.float32)
    att_src = np.asarray(att_src, dtype=np.float32)
    att_dst = np.asarray(att_dst, dtype=np.float32)
    bias = np.asarray(bias, dtype=np.float32)
    n_nodes = x.shape[0]

    meta = _preprocess(edge_index, n_nodes)
    lists, sc_offs = _build_gather_lists(meta)
    calls, gidx, gc16 = _plan_calls(meta, lists)

    cfg = dict(slots=meta["slots"], scs=meta["scs"], sc_offs=sc_offs,
               D_A=meta["D_A"], D_B=meta["D_B"], calls=calls, gc16=gc16)
    nc = _build_nc(cfg)

    wext = np.zeros((P, WCOLS), np.float32)
    wext[:, 0:F_OUT] = W
    wext[:, F_OUT] = W @ att_src
    wext[:, F_OUT + 1] = W @ att_dst
    wextb = wext.astype(ml_dtypes.bfloat16)

    xT = np.zeros((P, TROWS), np.float32)
    m = meta["rank"] >= 0
    xT[:, meta["prank"][m]] = x[m].T
    xTb = xT.astype(ml_dtypes.bfloat16)

    biasb_h = np.tile(bias[None, :], (P, 1)).astype(np.float32)
    padrow_f32 = np.zeros(P // 2, dtype=np.float32)
    padrow_f32[F_OUT // 2] = PAD_ASRC
    padrow_h = padrow_f32.view(ml_dtypes.bfloat16).reshape(1, P).copy()

    gmax = max(gc16, 16)
    in_maps = []
    for k in range(NCORES):
        ox = np.zeros((P, meta["slots"] * P), np.float32)
        nd = meta["node_at"][k].reshape(-1)
        mv = nd >= 0
        ox[:, mv] = x[nd[mv]].T
        gi = gidx[k]
        if gi.shape[1] < gmax:
            gi = np.concatenate(
                [gi, np.zeros((P, gmax - gi.shape[1]), np.int16)], axis=1)
        in_maps.append({
            "xTb": xTb, "wextb": wextb,
            "ownxt": ox.astype(ml_dtypes.bfloat16),
            "gidx": np.ascontiguousarray(gi),
            "biasb": biasb_h,
            "padrow": padrow_h,
        })

    res = run_bass_kernel_spmd(nc, in_maps, core_ids=list(range(NCORES)),
                               **_RUN_KW)
    _LAST_RESULT[0] = res

    out = np.zeros((n_nodes, F_OUT), dtype=np.float32)
    for k in range(NCORES):
        nd = meta["node_at"][k].reshape(-1)
        mv = nd >= 0
        out[nd[mv]] = res.results[k]["out"][mv]
    return out


# revision 33
# speedup vs baseline: 1.1453x; 1.1453x over previous
"""GAT layer (single head, PyG GATConv semantics + relu) on 8 Trainium2 cores.

Strategy (destination-major, descriptor-minimized):
  * ONE feature table in HBM per core (replicated build): row r = 256B =
    [h bf16 x64 | a_src f32 | pad], rows ordered by FIRST USE so each
    superchunk's gathers only need a table PREFIX -> the table build (phase
    A) overlaps the edge gathers (phase B).  Rows 0 and MID=17408 are pad
    rows (a_src = -1e4 => exp() = 0 for padded edge slots).
  * Two gather windows (int16 indices span 32768 rows): A = rows [0, 32768),
    B = rows [17408, 50176).  Rows in the overlap may be fetched by either
    call; per-edge assignment balances each destination's A/B column counts
    (near-zero split padding).  Each window's columns are further split into
    ~14-col calls spread over the 4 SWDGE queues.
  * Destinations sorted by (dA, dB) (fixed-point), dealt node-round-robin to
    cores so per-slot shapes are SPMD-uniform and tight.
  * Self-loops are folded in locally from ownxt (never gathered).
  * Per-slot compute: exp via per-partition-bias activation is replaced by
    superchunk-fused ops (z-add, 2 exps, max, cast, weighted multiply) with
    only the segment reductions per slot.
"""

import ml_dtypes
import numpy as np

import concourse.bass as bass
import concourse.tile as tile
from concourse import bacc, mybir
from concourse.bass_utils import run_bass_kernel_spmd

P = 128
NCORES = 8
TROWS = 50176
MID = 17408
WINB = 32768
NEG_SLOPE = 0.2
PAD_ASRC = -1.0e4
F_OUT = 64
WCOLS = F_OUT + 2


# --------------------------------------------------------------------------
# host-side preprocessing
# --------------------------------------------------------------------------

def _preprocess(edge_index, n_nodes, n_iter=6, cmax=56):
    src = np.asarray(edge_index[0], dtype=np.int64)
    dst = np.asarray(edge_index[1], dtype=np.int64)
    deg = np.bincount(dst, minlength=n_nodes)
    slots = (n_nodes + P * NCORES - 1) // (P * NCORES)

    order = np.argsort(-deg, kind="stable")
    for it in range(n_iter):
        pos = np.empty(n_nodes, np.int64)
        pos[order] = np.arange(n_nodes)
        slot_of = pos // (P * NCORES)
        e_slot = slot_of[dst]
        first = np.full(n_nodes, slots, np.int64)
        np.minimum.at(first, src, e_slot)
        used = first < slots
        uorder = np.argsort(first[used], kind="stable")
        un = used.nonzero()[0][uorder]
        rank = np.full(n_nodes, -1, np.int64)
        rank[un] = np.arange(len(un))
        prank = np.where(rank < MID - 1, rank + 1, rank + 2)
        # group-wise partition-major row remap: build writes 2KB-contiguous
        # runs of 8 rows per partition; 1024-aligned windows are preserved.
        pl = prank % 1024
        pos_t = (prank // 1024) * 1024 + (pl % 128) * 8 + pl // 128
        r = pos_t[src]
        onlyA = r < MID + 1
        onlyB = r >= WINB
        nA = np.bincount(dst[onlyA], minlength=n_nodes)
        nB = np.bincount(dst[onlyB], minlength=n_nodes)
        nM = deg - nA - nB
        tgt = (deg + 1) // 2
        dA = np.clip(tgt, nA, nA + nM)
        dB = deg - dA
        if it < n_iter - 1:
            order = np.lexsort((dB, dA))[::-1].copy()

    cap = slots * NCORES * P
    na = np.full(cap, -1, np.int64)
    na[:n_nodes] = order
    # sorted node r -> core r%8, slot r//1024, p (r//8)%128
    node_at = np.transpose(na.reshape(slots, P, NCORES), (2, 0, 1)).copy()

    da = np.zeros(cap, np.int64)
    db = np.zeros(cap, np.int64)
    da[:n_nodes] = dA[order]
    db[:n_nodes] = dB[order]
    D_A = da.reshape(slots, -1).max(axis=1)
    D_B = db.reshape(slots, -1).max(axis=1)

    prefA = np.full(slots, 2, np.int64)
    prefB = np.full(slots, MID + 2, np.int64)
    mA = r < WINB
    np.maximum.at(prefA, e_slot[mA], r[mA] + 1)
    mB = r >= MID + 1
    np.maximum.at(prefB, e_slot[mB], r[mB] + 1)
    prefA = np.maximum.accumulate(np.minimum(prefA, WINB))
    prefB = np.maximum.accumulate(prefB)
    prefA = np.minimum(-(-prefA // 1024) * 1024, WINB)
    prefB = np.minimum(-(-prefB // 1024) * 1024, TROWS)

    scs = []
    cur, cur_c = [], 0
    for s in range(slots):
        c = int(D_A[s] + D_B[s])
        single = s < 4 or s >= slots - 2
        if cur and (single or cur_c + c > cmax):
            scs.append(cur)
            cur, cur_c = [], 0
        cur.append(s)
        cur_c += c
        if single:
            scs.append(cur)
            cur, cur_c = [], 0
    if cur:
        scs.append(cur)

    return dict(
        deg=deg, dA=dA, dB=dB, order=order, pos=pos, pos_t=pos_t, rank=rank,
        prank=prank, node_at=node_at, slots=slots, D_A=D_A, D_B=D_B,
        prefA=prefA, prefB=prefB, scs=scs, src=src, dst=dst,
        n_nodes=n_nodes,
    )


def _build_gather_lists(meta):
    """Per (core, sc): (gA [P, colsA], gB [P, colsB]) window-local rows."""
    src, dst, pos = meta["src"], meta["dst"], meta["pos"]
    pos_t = meta["pos_t"]
    dA = meta["dA"]
    D_A, D_B = meta["D_A"], meta["D_B"]
    slots = meta["slots"]
    n_nodes = meta["n_nodes"]

    r_node = pos[dst]
    core_of = r_node % NCORES
    sp = r_node // NCORES
    slot_of = sp // P
    p_of = sp % P

    r = pos_t[src]
    onlyB = r >= WINB
    onlyA = r < MID + 1
    midm = ~onlyA & ~onlyB

    eo = np.lexsort((np.arange(len(src)), dst))
    is_mid = midm[eo]
    dsts = dst[eo]
    midrank = np.zeros(len(eo), np.int64)
    key = dsts[is_mid]
    grp_start = np.zeros(n_nodes + 1, np.int64)
    np.add.at(grp_start[1:], key, 1)
    np.cumsum(grp_start, out=grp_start)
    midrank[is_mid] = np.arange(is_mid.sum()) - grp_start[key]
    nA_map = np.bincount(dst[onlyA], minlength=n_nodes)
    quota = dA - nA_map
    toA = np.zeros(len(eo), bool)
    toA[~is_mid] = onlyA[eo][~is_mid]
    toA[is_mid] = midrank[is_mid] < quota[key]

    ek = core_of[eo]
    es = slot_of[eo]
    ep = p_of[eo]
    er = r[eo]
    skey = dsts * 2 + (~toA).astype(np.int64)
    sord = np.lexsort((np.arange(len(eo)), skey))
    _, first_idx, counts = np.unique(skey[sord], return_index=True,
                                     return_counts=True)
    jj = np.empty(len(eo), np.int64)
    jj[sord] = np.arange(len(eo)) - np.repeat(first_idx, counts)

    sc_of_slot = np.empty(slots, np.int64)
    bi_of_slot = np.empty(slots, np.int64)
    for ci, sc in enumerate(meta["scs"]):
        for bi, s in enumerate(sc):
            sc_of_slot[s] = ci
            bi_of_slot[s] = bi
    sc_offs = []
    for sc in meta["scs"]:
        offA = np.concatenate([[0], np.cumsum(D_A[sc])])
        offB = np.concatenate([[0], np.cumsum(D_B[sc])])
        sc_offs.append((offA.astype(int), offB.astype(int)))

    eci = sc_of_slot[es]
    ebi = bi_of_slot[es]
    lists = {}
    for ci, sc in enumerate(meta["scs"]):
        offA, offB = sc_offs[ci]
        for k in range(NCORES):
            lists[(k, ci)] = (np.zeros((P, int(offA[-1])), np.int64),
                              np.zeros((P, int(offB[-1])), np.int64))
    colA_e = np.zeros(len(eo), np.int64)
    colB_e = np.zeros(len(eo), np.int64)
    for ci in range(len(meta["scs"])):
        offA, offB = sc_offs[ci]
        m = eci == ci
        colA_e[m] = offA[ebi[m]] + jj[m]
        colB_e[m] = offB[ebi[m]] + jj[m]
    for k in range(NCORES):
        mk = ek == k
        ma = mk & toA
        mb = mk & ~toA
        for ci in range(len(meta["scs"])):
            gA, gB = lists[(k, ci)]
            mm = ma & (eci == ci)
            gA[ep[mm], colA_e[mm]] = er[mm]
            mm = mb & (eci == ci)
            gB[ep[mm], colB_e[mm]] = er[mm] - MID
    return lists, sc_offs


def _wrap_idx(arr):
    """dma_gather index layout: [128, n/16] int16, idx i at (i%16, i//16),
    replicated across the 8 Q7 core groups."""
    n = arr.shape[0]
    assert n % 16 == 0
    w = arr.reshape(n // 16, 16).T.astype(np.int16)
    return np.tile(w, (8, 1))


def _plan_calls(meta, lists, chunk_cols=14):
    """Split each sc's A/B column ranges into calls, balance over 4 queues.

    Returns: calls (list of dicts), gidx per core [P, gc16]."""
    calls = []
    qload = [0, 0, 0, 0]
    off16 = 0
    gidx = [[] for _ in range(NCORES)]
    for ci, sc in enumerate(meta["scs"]):
        gA0, gB0 = lists[(0, ci)]
        for side, cols, pref in (
            (0, gA0.shape[1], int(meta["prefA"][sc[-1]])),
            (1, gB0.shape[1], int(meta["prefB"][sc[-1]])),
        ):
            if cols == 0:
                continue
            nch = max(1, -(-cols // chunk_cols))
            bounds = np.linspace(0, cols, nch + 1).astype(int)
            for c0, c1 in zip(bounds[:-1], bounds[1:]):
                if c1 == c0:
                    continue
                q = min(range(4), key=lambda i: qload[i])
                qload[q] += c1 - c0
                ln16 = (c1 - c0) * P // 16
                for k in range(NCORES):
                    g = lists[(k, ci)][side]
                    gidx[k].append(_wrap_idx(g[:, c0:c1].T.ravel()))
                calls.append(dict(ci=ci, side=side, c0=int(c0), c1=int(c1),
                                  pref=pref, q=q, off16=off16, ln16=ln16))
                off16 += ln16
    gidx = [np.concatenate(g, axis=1) if g else np.zeros((P, 16), np.int16)
            for g in gidx]
    return calls, gidx, off16


# --------------------------------------------------------------------------
# device program
# --------------------------------------------------------------------------

def _build_nc(cfg):
    slots = cfg["slots"]
    scs = cfg["scs"]
    sc_offs = cfg["sc_offs"]
    D_A, D_B = cfg["D_A"], cfg["D_B"]
    calls = cfg["calls"]
    gc16 = max(cfg["gc16"], 16)

    fp32 = mybir.dt.float32
    bf16 = mybir.dt.bfloat16

    nc = bacc.Bacc("TRN2", target_bir_lowering=False, debug=False,
                   num_devices=NCORES, num_swdge_queues=4)
    xTb = nc.dram_tensor("xTb", [P, TROWS], bf16, kind="ExternalInput")
    wextb = nc.dram_tensor("wextb", [P, WCOLS], bf16, kind="ExternalInput")
    ownxt = nc.dram_tensor("ownxt", [P, slots * P], bf16,
                           kind="ExternalInput")
    gidx_d = nc.dram_tensor("gidx", [P, gc16], mybir.dt.int16,
                            kind="ExternalInput")
    biasb = nc.dram_tensor("biasb", [P, F_OUT], fp32, kind="ExternalInput")
    padrow = nc.dram_tensor("padrow", [1, P], bf16, kind="ExternalInput")
    out_d = nc.dram_tensor("out", [slots * P, F_OUT], fp32,
                           kind="ExternalOutput")
    tbl = nc.dram_tensor("tbl", [TROWS, P], bf16, kind="Internal")

    with tile.TileContext(nc) as tc:
        with (
            tc.tile_pool(name="const", bufs=1) as cpool,
            tc.tile_pool(name="xt", bufs=6) as xtpool,
            tc.tile_pool(name="ps", bufs=7, space="PSUM") as pspool,
            tc.tile_pool(name="tstage", bufs=6) as tspool,
            tc.tile_pool(name="gat", bufs=5) as gpool,
            tc.tile_pool(name="wgt", bufs=3) as wpool,
            tc.tile_pool(name="sc", bufs=3) as scpool,
            tc.tile_pool(name="blk", bufs=3) as bpool,
        ):
            wextb_sb = cpool.tile([P, WCOLS], bf16)
            nc.sync.dma_start(out=wextb_sb[:], in_=wextb[:])
            biasb_sb = cpool.tile([P, F_OUT], fp32)
            nc.sync.dma_start(out=biasb_sb[:], in_=biasb[:])
            ownxt_sb = cpool.tile([P, slots * P], bf16)
            gidx_sb = cpool.tile([P, gc16], mybir.dt.int16)

            h_own = cpool.tile([P, slots, F_OUT], bf16)
            aos = cpool.tile([P, slots], fp32)
            aod = cpool.tile([P, slots], fp32)

            def own_precompute():
                for i0 in range(0, slots, 4):
                    hn = min(4, slots - i0)
                    ps2 = pspool.tile([P, 4, WCOLS], fp32, tag="mm")
                    for j in range(hn):
                        nc.tensor.matmul(
                            out=ps2[:, j, :],
                            lhsT=ownxt_sb[:, (i0 + j) * P:(i0 + j + 1) * P],
                            rhs=wextb_sb[:], start=True, stop=True)
                    if (i0 // 4) % 2 == 0:
                        nc.scalar.copy(out=h_own[:, i0:i0 + hn, :],
                                       in_=ps2[:, 0:hn, 0:F_OUT])
                    else:
                        nc.vector.tensor_copy(out=h_own[:, i0:i0 + hn, :],
                                              in_=ps2[:, 0:hn, 0:F_OUT])
                    nc.vector.tensor_copy(
                        out=aos[:, i0:i0 + hn],
                        in_=ps2[:, 0:hn, F_OUT:F_OUT + 1].rearrange(
                            "p h one -> p (h one)"))
                    nc.vector.tensor_copy(
                        out=aod[:, i0:i0 + hn],
                        in_=ps2[:, 0:hn, F_OUT + 1:F_OUT + 2].rearrange(
                            "p h one -> p (h one)"))
                zown = cpool.tile([P, slots], fp32)
                nc.vector.tensor_tensor(out=zown[:], in0=aos[:], in1=aod[:],
                                        op=mybir.AluOpType.add)
                e1o = cpool.tile([P, slots], fp32)
                e2o = cpool.tile([P, slots], fp32)
                nc.scalar.activation(out=e1o[:], in_=zown[:],
                                     func=mybir.ActivationFunctionType.Exp,
                                     scale=1.0)
                nc.scalar.activation(out=e2o[:], in_=zown[:],
                                     func=mybir.ActivationFunctionType.Exp,
                                     scale=NEG_SLOPE)
                s_ii = cpool.tile([P, slots], fp32)
                nc.vector.tensor_tensor(out=s_ii[:], in0=e1o[:], in1=e2o[:],
                                        op=mybir.AluOpType.max)
                s_ii16 = cpool.tile([P, slots], bf16)
                nc.vector.tensor_copy(out=s_ii16[:], in_=s_ii[:])
                aod02 = cpool.tile([P, slots], fp32)
                nc.vector.tensor_scalar_mul(aod02[:], aod[:], NEG_SLOPE)
                return s_ii, s_ii16, aod02

            # ---- phase A: table build (first-use row order), own-node
            # precompute interleaved after the early prefix groups ----
            WB = 8
            nblk_tbl = TROWS // P
            own_result = [None]
            for g0 in range(0, nblk_tbl, WB):
                gn = min(WB, nblk_tbl - g0)
                xtb8 = xtpool.tile([P, WB, P], bf16, tag="xtb")
                nc.sync.dma_start(
                    out=xtb8[:, 0:gn, :],
                    in_=xTb[:, g0 * P:(g0 + gn) * P].rearrange(
                        "p (i q) -> p i q", q=P))
                tstage = tspool.tile([P, WB, P], bf16)
                for h0 in range(0, gn, 4):
                    hn = min(4, gn - h0)
                    ps4 = pspool.tile([P, 4, WCOLS], fp32, tag="mm")
                    for bi in range(hn):
                        nc.tensor.matmul(out=ps4[:, bi, :],
                                         lhsT=xtb8[:, h0 + bi, :].squeeze(),
                                         rhs=wextb_sb[:],
                                         start=True, stop=True)
                    if h0 == 0:
                        nc.scalar.copy(out=tstage[:, h0:h0 + hn, 0:F_OUT],
                                       in_=ps4[:, 0:hn, 0:F_OUT])
                    else:
                        nc.vector.tensor_copy(out=tstage[:, h0:h0 + hn, 0:F_OUT],
                                              in_=ps4[:, 0:hn, 0:F_OUT])
                    nc.vector.tensor_copy(
                        out=tstage[:, h0:h0 + hn, F_OUT:F_OUT + 4].bitcast(fp32),
                        in_=ps4[:, 0:hn, F_OUT:F_OUT + 2])
                r0, r1 = g0 * P, (g0 + gn) * P
                # partition-major rows: row r0 + p*gn + i <- tstage[p, i, :]
                nc.scalar.dma_start(
                    out=tbl[r0:r1, :].rearrange("(p i) w -> p i w", i=gn),
                    in_=tstage[:, 0:gn, :])
                if r0 == 0:
                    nc.sync.dma_start(out=tbl[0:1, :], in_=padrow[:])
                if r0 <= MID < r1:
                    nc.sync.dma_start(out=tbl[MID:MID + 1, :], in_=padrow[:])
                if g0 == WB:
                    nc.sync.dma_start(out=gidx_sb[:], in_=gidx_d[:])
                if g0 == 18 * WB:
                    nc.sync.dma_start(out=ownxt_sb[:], in_=ownxt[:])
                if g0 == 21 * WB:
                    own_result[0] = own_precompute()
            s_ii, s_ii16, aod02 = own_result[0]

            # ---- phase B ----
            calls_by_sc = {}
            for cl in calls:
                calls_by_sc.setdefault(cl["ci"], []).append(cl)

            for ci, sc in enumerate(scs):
                offA, offB = sc_offs[ci]
                cA, cB = int(offA[-1]), int(offB[-1])
                ncols = cA + cB
                nb = len(sc)
                i0 = sc[0]
                g_t = gpool.tile([P, ncols, P], bf16)
                for cl in calls_by_sc[ci]:
                    base = cl["c0"] + (cA if cl["side"] else 0)
                    n_i = (cl["c1"] - cl["c0"]) * P
                    in_ap = (tbl[0:cl["pref"], :] if cl["side"] == 0
                             else tbl[MID:cl["pref"], :])
                    nc.gpsimd.dma_gather(
                        out_ap=g_t[:, base:base + (cl["c1"] - cl["c0"]), :],
                        in_ap=in_ap,
                        idxs_ap=gidx_sb[:, cl["off16"]:cl["off16"] + cl["ln16"]],
                        num_idxs=n_i, num_idxs_reg=n_i,
                        elem_size=P, single_packet=False, queue_num=cl["q"])

                # per (slot, half): exp with per-partition bias on ACT, then
                # fused max + bf16 cast + denominator-accumulate on DVE
                e1 = scpool.tile([P, ncols], fp32, tag="e1")
                e2 = scpool.tile([P, ncols], fp32, tag="e2")
                s16 = scpool.tile([P, ncols], bf16, tag="s16")
                dn = bpool.tile([P, 2 * nb], fp32, tag="dn")
                asrc_v = g_t[:, :, F_OUT:F_OUT + 2].bitcast(fp32)
                for bi, s in enumerate(sc):
                    for hi, (c0, c1) in enumerate(
                        ((int(offA[bi]), int(offA[bi + 1])),
                         (cA + int(offB[bi]), cA + int(offB[bi + 1])))):
                        j = 2 * bi + hi
                        if c1 > c0:
                            av = asrc_v[:, c0:c1, :].rearrange(
                                "p c one -> p (c one)")
                            nc.scalar.activation(
                                out=e1[:, c0:c1], in_=av,
                                func=mybir.ActivationFunctionType.Exp,
                                bias=aod[:, s:s + 1], scale=1.0)
                            nc.scalar.activation(
                                out=e2[:, c0:c1], in_=av,
                                func=mybir.ActivationFunctionType.Exp,
                                bias=aod02[:, s:s + 1], scale=NEG_SLOPE)
                            nc.vector.scalar_tensor_tensor(
                                out=s16[:, c0:c1], in0=e1[:, c0:c1],
                                scalar=1.0, in1=e2[:, c0:c1],
                                op0=mybir.AluOpType.bypass,
                                op1=mybir.AluOpType.max,
                                accum_out=dn[:, j:j + 1])
                        else:
                            nc.vector.memset(dn[:, j:j + 1], 0.0)
                wgt = wpool.tile([P, ncols, F_OUT], bf16)
                nc.vector.tensor_tensor(
                    out=wgt[:], in0=g_t[:, :, 0:F_OUT],
                    in1=s16[:].unsqueeze(2).broadcast_to([P, ncols, F_OUT]),
                    op=mybir.AluOpType.mult)

                wsA = bpool.tile([P, nb, F_OUT], fp32, tag="wsA")
                wsB = bpool.tile([P, nb, F_OUT], fp32, tag="wsB")
                for bi, s in enumerate(sc):
                    for ws, (c0, c1) in (
                        (wsA, (int(offA[bi]), int(offA[bi + 1]))),
                        (wsB, (cA + int(offB[bi]), cA + int(offB[bi + 1])))):
                        if c1 > c0:
                            nc.vector.tensor_reduce(
                                out=ws[:, bi, :],
                                in_=wgt[:, c0:c1, :].rearrange(
                                    "p c f -> p f c"),
                                axis=mybir.AxisListType.X,
                                op=mybir.AluOpType.add)
                        else:
                            nc.vector.memset(ws[:, bi, :], 0.0)

                den = bpool.tile([P, nb], fp32, tag="den")
                nc.vector.tensor_reduce(
                    out=den[:], in_=dn[:].rearrange("p (b t) -> p b t", t=2),
                    axis=mybir.AxisListType.X, op=mybir.AluOpType.add)
                nc.vector.tensor_add(den[:], den[:], s_ii[:, i0:i0 + nb])
                rec = bpool.tile([P, nb], fp32, tag="rec")
                nc.vector.reciprocal(rec[:], den[:])

                num = bpool.tile([P, nb, F_OUT], fp32, tag="num")
                nc.vector.tensor_add(num[:], wsA[:], wsB[:])
                selfm = bpool.tile([P, nb, F_OUT], fp32, tag="selfm")
                nc.vector.tensor_tensor(
                    out=selfm[:], in0=h_own[:, i0:i0 + nb, :],
                    in1=s_ii16[:, i0:i0 + nb].unsqueeze(2).broadcast_to(
                        [P, nb, F_OUT]),
                    op=mybir.AluOpType.mult)
                nc.vector.tensor_add(num[:], num[:], selfm[:])
                nc.vector.tensor_tensor(
                    out=num[:], in0=num[:],
                    in1=rec[:].unsqueeze(2).broadcast_to([P, nb, F_OUT]),
                    op=mybir.AluOpType.mult)
                nc.vector.tensor_tensor(
                    out=num[:], in0=num[:],
                    in1=biasb_sb[:].unsqueeze(1).broadcast_to([P, nb, F_OUT]),
                    op=mybir.AluOpType.add)
                ostage = bpool.tile([P, nb, F_OUT], fp32, tag="ostage")
                nc.scalar.activation(out=ostage[:], in_=num[:],
                                     func=mybir.ActivationFunctionType.Relu)
                nc.sync.dma_start(
                    out=out_d[i0 * P:(i0 + nb) * P, :].rearrange(
                        "(i p) f -> p i f", p=P),
                    in_=ostage[:])
    nc.compile()
    return nc


# --------------------------------------------------------------------------
# entry point
# --------------------------------------------------------------------------

_RUN_KW = {}
_LAST_RESULT = [None]


def kernel(x, edge_index, W, att_src, att_dst, bias):
    x = np.asarray(x, dtype=np.float32)
    W = np.asarray(W, dtype=np.float32)
    att_src = np.asarray(att_src, dtype=np.float32)
    att_dst = np.asarray(att_dst, dtype=np.float32)
    bias = np.asarray(bias, dtype=np.float32)
    n_nodes = x.shape[0]

    meta = _preprocess(edge_index, n_nodes)
    lists, sc_offs = _build_gather_lists(meta)
    calls, gidx, gc16 = _plan_calls(meta, lists)

    cfg = dict(slots=meta["slots"], scs=meta["scs"], sc_offs=sc_offs,
               D_A=meta["D_A"], D_B=meta["D_B"], calls=calls, gc16=gc16)
    nc = _build_nc(cfg)

    wext = np.zeros((P, WCOLS), np.float32)
    wext[:, 0:F_OUT] = W
    wext[:, F_OUT] = W @ att_src
    wext[:, F_OUT + 1] = W @ att_dst
    wextb = wext.astype(ml_dtypes.bfloat16)

    xT = np.zeros((P, TROWS), np.float32)
    m = meta["rank"] >= 0
    xT[:, meta["prank"][m]] = x[m].T
    xTb = xT.astype(ml_dtypes.bfloat16)

    biasb_h = np.tile(bias[None, :], (P, 1)).astype(np.float32)
    padrow_f32 = np.zeros(P // 2, dtype=np.float32)
    padrow_f32[F_OUT // 2] = PAD_ASRC
    padrow_h = padrow_f32.view(ml_dtypes.bfloat16).reshape(1, P).copy()

    gmax = max(gc16, 16)
    in_maps = []
    for k in range(NCORES):
        ox = np.zeros((P, meta["slots"] * P), np.float32)
        nd = meta["node_at"][k].reshape(-1)
        mv = nd >= 0
        ox[:, mv] = x[nd[mv]].T
        gi = gidx[k]
        if gi.shape[1] < gmax:
            gi = np.concatenate(
                [gi, np.zeros((P, gmax - gi.shape[1]), np.int16)], axis=1)
        in_maps.append({
            "xTb": xTb, "wextb": wextb,
            "ownxt": ox.astype(ml_dtypes.bfloat16),
            "gidx": np.ascontiguousarray(gi),
            "biasb": biasb_h,
            "padrow": padrow_h,
        })

    res = run_bass_kernel_spmd(nc, in_maps, core_ids=list(range(NCORES)),
                               **_RUN_KW)
    _LAST_RESULT[0] = res

    out = np.zeros((n_nodes, F_OUT), dtype=np.float32)
    for k in range(NCORES):
        nd = meta["node_at"][k].reshape(-1)
        mv = nd >= 0
        out[nd[mv]] = res.results[k]["out"][mv]
    return out
